# revision 1
# baseline (speedup 1.0000x reference)
"""Multi-head attention (B=2, S=2048, D=1024, H=16) on 8 Trainium2 cores.

Sharding: core c = (batch b, head-group hg) with b = c // 4, hg = c % 4.
Each core computes 4 heads of one batch element end-to-end:
  - Q^T/K^T projections in [dh, s] layout (scores computed transposed so the
    softmax denominator comes out of the PV matmul via a ones-column in V)
  - V projection in natural [s, dh] layout
  - exp on ScalarE with the 1/sqrt(dh) scale fused into the activation
  - partial output projection against the core's row-slice of Wo
Host sums the 4 partial projections per batch and adds bo.

Matmuls run as float32r (full-rate fp32 path on the PE for moving dim >= 256);
accumulation is always fp32 in PSUM. Walrus requires fp32r matmul operands to
be produced by an instruction that rounds to fp32r, so every matmul input tile
is allocated with dtype float32r and written by a DVE/ACT op (the rounding is
fused into copies we need anyway). Input transposes run in plain fp32.
"""

import numpy as np

import concourse.bacc as bacc
import concourse.mybir as mybir
import concourse.tile as tile
from concourse.bass_utils import run_bass_kernel_spmd
from concourse.masks import make_identity

F32 = mybir.dt.float32
F32R = mybir.dt.float32r

S_FULL, D_FULL, NH_PER_CORE, DH = 2048, 1024, 4, 64
N_CORES = 8
B_FULL, H_FULL = 2, 16


def build_core_program(S=S_FULL, D=D_FULL, NH=NH_PER_CORE):
    """One core's program: inputs xq/xk/xv [S,D], weight slices wq/wk/wv
    [D,NSL], wo [NSL,D], biases [NSL]; output out [S,D] (partial sum)."""
    NSL = NH * DH            # projection slice width for this core
    P = 128
    KD = D // P              # d-tiles (contraction tiles for projections)
    NT = NSL // P            # n-tiles = head-pairs
    ST = S // P              # s-tiles
    SBLK = 512 if S % 512 == 0 else S
    NB = S // SBLK           # s/i blocks
    JT = ST                  # j-tiles in attention
    JC = 2                   # j-tiles per score/exp chunk
    SS = SBLK // P           # s-subtiles per block

    nc = bacc.Bacc("TRN2", target_bir_lowering=False, debug=False)

    xq_d = nc.dram_tensor("xq", [S, D], F32, kind="ExternalInput")
    xk_d = nc.dram_tensor("xk", [S, D], F32, kind="ExternalInput")
    xv_d = nc.dram_tensor("xv", [S, D], F32, kind="ExternalInput")
    wq_d = nc.dram_tensor("wq", [D, NSL], F32, kind="ExternalInput")
    wk_d = nc.dram_tensor("wk", [D, NSL], F32, kind="ExternalInput")
    wv_d = nc.dram_tensor("wv", [D, NSL], F32, kind="ExternalInput")
    wo_d = nc.dram_tensor("wo", [NSL, D], F32, kind="ExternalInput")
    bq_d = nc.dram_tensor("bq", [NSL], F32, kind="ExternalInput")
    bk_d = nc.dram_tensor("bk", [NSL], F32, kind="ExternalInput")
    bv_d = nc.dram_tensor("bv", [NSL], F32, kind="ExternalInput")
    out_d = nc.dram_tensor("out", [S, D], F32, kind="ExternalOutput")

    with tile.TileContext(nc) as tc:
        with tc.tile_pool(name="persist", bufs=1) as pp:
            ident = pp.tile([P, P], F32)
            make_identity(nc, ident)

            # Weights: DMA fp32 staging -> rounded fp32r copies.
            wq_sb = pp.tile([P, KD, NSL], F32R)
            wk_sb = pp.tile([P, KD, NSL], F32R)
            wv_sb = pp.tile([P, KD, NSL], F32R)
            wo_sb = pp.tile([P, NT, D], F32R)
            bq_sb = pp.tile([P, NT], F32)
            nc.sync.dma_start(bq_sb, bq_d.rearrange("(t p) -> p t", p=P))
            bk_sb = pp.tile([P, NT], F32)
            nc.sync.dma_start(bk_sb, bk_d.rearrange("(t p) -> p t", p=P))
            bv_sb = pp.tile([P, NT], F32)
            nc.sync.dma_start(bv_sb, bv_d.rearrange("(t p) -> p t", p=P))

            with tc.tile_pool(name="wstage", bufs=2) as wsp:
                for w_d, w_sb, wkd, wn in (
                    (wq_d, wq_sb, KD, NSL),
                    (wk_d, wk_sb, KD, NSL),
                    (wv_d, wv_sb, KD, NSL),
                    (wo_d, wo_sb, NT, D),
                ):
                    wst = wsp.tile([P, wkd, wn], F32, tag="wst")
                    nc.sync.dma_start(
                        wst, w_d.rearrange("(t p) n -> p t n", p=P)
                    )
                    nc.vector.tensor_copy(w_sb, wst)

            # qT/o_cat are per-i-block tensors so attention / out-projection
            # dependencies stay block-granular (enables cross-phase overlap).
            qT_b = [
                pp.tile([P, NT, SBLK], F32R, name=f"qT{b}") for b in range(NB)
            ]
            kT = pp.tile([P, NT, S], F32R)
            ones_colf = pp.tile([1, DH], F32)
            nc.vector.memset(ones_colf, 1.0)
            ones_col = pp.tile([1, DH], F32R)
            nc.vector.tensor_copy(ones_col, ones_colf)
            v_sb = pp.tile([P, JT, NH, DH + 1], F32R)  # natural V + ones col
            vonesf = pp.tile([P, JT, NH, 1], F32)
            nc.vector.memset(vonesf, 1.0)
            nc.vector.tensor_copy(v_sb[:, :, :, DH : DH + 1], vonesf)
            o_b = [
                pp.tile([P, NT, SBLK], F32R, name=f"o{b}") for b in range(NB)
            ]

            # ---- Phase A: transpose inputs + projections ----
            with tc.tile_pool(name="pha", bufs=2) as pa, \
                 tc.tile_pool(name="psa", bufs=2, space="PSUM") as psa:
                plans = [
                    (xv_d, wv_sb, None, None, "v"),
                    (xk_d, wk_sb, bk_sb, None, "qk"),
                    (xq_d, wq_sb, bq_sb, qT_b, "q"),
                ]
                for x_d, w_sb, b_sb, dstT, kind in plans:
                    for blk in range(NB):
                        xn = pa.tile([P, SS, D], F32, tag="xn")
                        nc.sync.dma_start(
                            xn,
                            x_d[blk * SBLK : (blk + 1) * SBLK].rearrange(
                                "(ss p) d -> p ss d", p=P
                            ),
                        )
                        xT = pa.tile([P, KD, SBLK], F32R, tag="xT")
                        for ss in range(SS):
                            for kd in range(KD):
                                pst = psa.tile([P, P], F32, tag="pst", bufs=4)
                                nc.tensor.transpose(
                                    pst,
                                    xn[:, ss, kd * P : (kd + 1) * P],
                                    ident,
                                )
                                # split casts across DVE and the (otherwise
                                # idle in this phase) scalar engine
                                dst_sl = xT[:, kd, ss * P : (ss + 1) * P]
                                if (ss * KD + kd) % 2 == 0:
                                    nc.vector.tensor_copy(dst_sl, pst)
                                else:
                                    nc.scalar.copy(dst_sl, pst)
                        if kind in ("qk", "q"):
                            for nt in range(NT):
                                psp = psa.tile([P, SBLK], F32, tag="psp")
                                for kd in range(KD):
                                    nc.tensor.matmul(
                                        psp,
                                        lhsT=w_sb[:, kd, nt * P : (nt + 1) * P],
                                        rhs=xT[:, kd, :],
                                        start=(kd == 0),
                                        stop=(kd == KD - 1),
                                    )
                                dst = (
                                    dstT[blk][:, nt, :]
                                    if kind == "q"
                                    else kT[:, nt, blk * SBLK : (blk + 1) * SBLK]
                                )
                                nc.vector.tensor_scalar_add(
                                    dst, psp, b_sb[:, nt : nt + 1]
                                )
                        else:
                            for ss in range(SS):
                                psv = psa.tile([P, NSL], F32, tag="psv")
                                for kd in range(KD):
                                    nc.tensor.matmul(
                                        psv,
                                        lhsT=xT[:, kd, ss * P : (ss + 1) * P],
                                        rhs=wv_sb[:, kd, :],
                                        start=(kd == 0),
                                        stop=(kd == KD - 1),
                                    )
                                st = blk * SS + ss
                                nc.vector.tensor_copy(
                                    v_sb[:, st, :, 0:DH],
                                    psv.rearrange("p (h d) -> p h d", d=DH),
                                )

            # ---- Phase B: attention per i-block, per head-pair; the output
            # projection for each finished i-block is fused in as dense PE
            # filler (keeps the HAM clock warm through the ACT-paced chunks).
            with tc.tile_pool(name="phb", bufs=2) as pb, \
                 tc.tile_pool(name="psb", bufs=1, space="PSUM") as psb:
                for ib in range(NB):
                    for hp in range(NT):
                        ps_o = [
                            psb.tile([P, SBLK], F32, tag=f"ps_o{h01}",
                                     bufs=1, name=f"ps_o{h01}")
                            for h01 in range(2)
                        ]

                        def emit_exp_pv(jc, ps_s):
                            for h01 in range(2):
                                h = hp * 2 + h01
                                p_sb = pb.tile([P, JC, SBLK], F32R,
                                               tag=f"p_sb{h01}", bufs=4,
                                               name="p_sb")
                                nc.scalar.activation(
                                    p_sb, ps_s[h01],
                                    mybir.ActivationFunctionType.Exp,
                                    scale=float(1.0 / np.sqrt(DH)),
                                )
                                for jj in range(JC):
                                    jt = jc * JC + jj
                                    nc.tensor.matmul(
                                        ps_o[h01][0 : DH + 1, :],
                                        lhsT=v_sb[:, jt, h, :],
                                        rhs=p_sb[:, jj, :],
                                        start=(jt == 0),
                                        stop=(jt == JT - 1),
                                    )

                        prev = None
                        for jc in range(JT // JC):
                            ps_s = [
                                psb.tile([P, JC, SBLK], F32, tag="ps_s",
                                         bufs=3, name=f"ps_s{h01}")
                                for h01 in range(2)
                            ]
                            for jj in range(JC):
                                jt = jc * JC + jj
                                for h01 in range(2):
                                    base = h01 * DH
                                    nc.tensor.matmul(
                                        ps_s[h01][:, jj, :],
                                        lhsT=kT[base : base + DH, hp,
                                                jt * P : (jt + 1) * P],
                                        rhs=qT_b[ib][base : base + DH, hp, :],
                                        start=True,
                                        stop=True,
                                        tile_position=(base, 0),
                                    )
                            if prev is not None:
                                emit_exp_pv(*prev)
                            prev = (jc, ps_s)
                        emit_exp_pv(*prev)
                        for h01 in range(2):
                            base = h01 * DH
                            recf = pb.tile([1, SBLK], F32, tag="recf", bufs=2)
                            nc.vector.reciprocal(recf, ps_o[h01][DH : DH + 1, :])
                            rec = pb.tile([1, SBLK], F32R, tag="rec", bufs=2)
                            nc.vector.tensor_copy(rec, recf)
                            ps_b = psb.tile([P, JC, SBLK], F32, tag="ps_s",
                                            bufs=3, name="ps_b")[0:DH, 0, :]
                            nc.tensor.matmul(
                                ps_b, lhsT=ones_col, rhs=rec,
                                start=True, stop=True,
                            )
                            bc = pb.tile([DH, SBLK], F32, tag="bc", bufs=2)
                            nc.vector.tensor_copy(bc, ps_b)
                            o_slice = o_b[ib][base : base + DH, hp, :]
                            nc.vector.tensor_mul(o_slice, ps_o[h01][0:DH, :], bc)
                            nc.vector.tensor_scalar_add(
                                o_slice, o_slice,
                                bv_sb[base : base + DH, hp : hp + 1],
                            )

                    # output projection for this finished i-block
                    for st in range(ib * SS, (ib + 1) * SS):
                        for nb in range(D // SBLK):
                            pso = psb.tile([P, JC, SBLK], F32, tag="ps_s",
                                           bufs=3, name="pso")[:, 0, :]
                            for t in range(NT):
                                ss_off = (st - ib * SS) * P
                                nc.tensor.matmul(
                                    pso,
                                    lhsT=o_b[ib][:, t, ss_off : ss_off + P],
                                    rhs=wo_sb[:, t, nb * SBLK : (nb + 1) * SBLK],
                                    start=(t == 0),
                                    stop=(t == NT - 1),
                                )
                            ob = pb.tile([P, SBLK], F32, tag="ob", bufs=3)
                            nc.vector.tensor_copy(ob, pso)
                            nc.sync.dma_start(
                                out_d[st * P : (st + 1) * P,
                                      nb * SBLK : (nb + 1) * SBLK],
                                ob,
                            )

    nc.finalize()
    return nc


_NC_CACHE = {}


def _get_program(S, D, NH):
    key = (S, D, NH)
    if key not in _NC_CACHE:
        _NC_CACHE[key] = build_core_program(S, D, NH)
    return _NC_CACHE[key]


def kernel(q, k, v, Wq, bq, Wk, bk, Wv, bv, Wo, bo):
    q, k, v = (np.asarray(x, np.float32) for x in (q, k, v))
    Wq, Wk, Wv, Wo = (np.asarray(x, np.float32) for x in (Wq, Wk, Wv, Wo))
    bq, bk, bv, bo = (np.asarray(x, np.float32) for x in (bq, bk, bv, bo))
    B, S, D = q.shape
    GROUPS = N_CORES // B
    NSL = D // GROUPS

    nc = _get_program(S, D, NSL // DH)

    in_maps = []
    for c in range(N_CORES):
        b, hg = c // GROUPS, c % GROUPS
        sl = slice(hg * NSL, (hg + 1) * NSL)
        in_maps.append(
            {
                "xq": q[b],
                "xk": k[b],
                "xv": v[b],
                "wq": np.ascontiguousarray(Wq[:, sl]),
                "wk": np.ascontiguousarray(Wk[:, sl]),
                "wv": np.ascontiguousarray(Wv[:, sl]),
                "wo": np.ascontiguousarray(Wo[sl, :]),
                "bq": np.ascontiguousarray(bq[sl]),
                "bk": np.ascontiguousarray(bk[sl]),
                "bv": np.ascontiguousarray(bv[sl]),
            }
        )

    res = run_bass_kernel_spmd(nc, in_maps, list(range(N_CORES)))

    out = np.zeros((B, S, D), np.float32)
    for c in range(N_CORES):
        b = c // GROUPS
        out[b] += res.results[c]["out"]
    out += bo[None, None, :]
    return out



# revision 4
# speedup vs baseline: 1.3478x; 1.3478x over previous
"""Multi-head attention (B=2, S=2048, D=1024, H=16) on 8 Trainium2 cores.

Sharding: core c = (batch b, head-group hg) with b = c // 4, hg = c % 4.
Each core computes 4 heads of one batch element end-to-end and emits a
partial output projection; the host sums the 4 partials per batch and adds
(bv @ Wo + bo) (the value-bias term commutes through the softmax since the
attention weights sum to 1).

v2 layout strategy (vs the fp32r baseline):
  - Host pre-transposes q/k/v to x^T [D, S] and pre-marshals every tensor
    into its exact SBUF layout ([128, ...] partition-major) in bf16, so all
    DMAs are contiguous-row streams and the 384 PE transposes + PSUM->SBUF
    cast copies disappear entirely.
  - All matmuls run bf16 (full-rate on the PE; accumulation fp32 in PSUM).
  - Softmax denominator comes from a ones-column in V via the PV matmul;
    1/Z is computed with the fast custom-DVE reciprocal straight out of
    PSUM, broadcast across partitions on the otherwise-idle GpSimd engine,
    and applied with one DVE multiply. No PE broadcast, no 3.3us DVE
    reciprocals.
  - Emission order pipelines the whole program: K proj -> Q0 proj ->
    V proj -> per-(i-block, head-pair) attention with in-group
    scores/exp/PV software pipelining, Q(i+1) and the i-block's output
    projection fused between attention groups. ACT streams exp nearly
    continuously from ~35us onward.
"""

import numpy as np
import ml_dtypes

import concourse.bacc as bacc
import concourse.mybir as mybir
import concourse.tile as tile
from concourse.bass_utils import run_bass_kernel_spmd

F32 = mybir.dt.float32
BF16 = mybir.dt.bfloat16
BF = ml_dtypes.bfloat16

S_FULL, D_FULL, NH_PER_CORE, DH = 2048, 1024, 4, 64
N_CORES = 8
B_FULL, H_FULL = 2, 16

P = 128
S, D, NH = S_FULL, D_FULL, NH_PER_CORE
NSL = NH * DH            # 256: projection slice width per core
KD = D // P              # 8 contraction tiles
NT = NSL // P            # 2 head-pairs
ST = S // P              # 16 s-tiles
SBLK = 512               # i-block width
NB = S // SBLK           # 4 i-blocks
JT = ST                  # 16 j-tiles
JC = 2                   # j-tiles per score/exp chunk
SH = S // 2              # half-sequence (DMA/pipeline granularity)


def build_core_program():
    nc = bacc.Bacc("TRN2", target_bir_lowering=False, debug=False)

    xq_d = [nc.dram_tensor(f"xq{h}", [P, KD, SH], BF16, kind="ExternalInput")
            for h in range(2)]
    xk_d = [nc.dram_tensor(f"xk{h}", [P, KD, SH], BF16, kind="ExternalInput")
            for h in range(2)]
    xv_d = [nc.dram_tensor(f"xv{h}", [P, KD, SH], BF16, kind="ExternalInput")
            for h in range(2)]
    wq_d = nc.dram_tensor("wq", [P, KD, NSL], BF16, kind="ExternalInput")
    wk_d = nc.dram_tensor("wk", [P, KD, NSL], BF16, kind="ExternalInput")
    wv_d = nc.dram_tensor("wv", [P, KD, NSL], BF16, kind="ExternalInput")
    wo_d = nc.dram_tensor("wo", [P, NT, D], BF16, kind="ExternalInput")
    bq_d = nc.dram_tensor("bq", [P, NT], F32, kind="ExternalInput")
    bk_d = nc.dram_tensor("bk", [P, NT], F32, kind="ExternalInput")
    out_d = nc.dram_tensor("out", [S, D], F32, kind="ExternalOutput")

    with tile.TileContext(nc) as tc:
        with tc.tile_pool(name="persist", bufs=1) as pp, \
             tc.tile_pool(name="work", bufs=2) as pw, \
             tc.tile_pool(name="pa", bufs=2, space="PSUM") as pa, \
             tc.tile_pool(name="pb", bufs=1, space="PSUM") as psb:

            # ---- persistent SBUF tensors ----
            wq_sb = pp.tile([P, KD, NSL], BF16, name="wq")
            wk_sb = pp.tile([P, KD, NSL], BF16, name="wk")
            wv_sb = pp.tile([P, KD, NSL], BF16, name="wv")
            wo_sb = pp.tile([P, NT, D], BF16, name="wo")
            bq_sb = pp.tile([P, NT], F32, name="bq")
            bk_sb = pp.tile([P, NT], F32, name="bk")
            xq_sb = [pp.tile([P, KD, SH], BF16, name=f"xq{h}") for h in range(2)]
            xk_sb = [pp.tile([P, KD, SH], BF16, name=f"xk{h}") for h in range(2)]
            xv_sb = [pp.tile([P, KD, SH], BF16, name=f"xv{h}") for h in range(2)]
            kT = [pp.tile([P, NT, SH], BF16, name=f"kT{h}") for h in range(2)]
            qT = [pp.tile([P, NT, SBLK], BF16, name=f"qT{b}") for b in range(NB)]
            # natural-layout V (+ ones column feeding the softmax denominator)
            v_sb = [pp.tile([P, JT // 2, NH, DH + 1], BF16, name=f"v{h}")
                    for h in range(2)]
            for h in range(2):
                nc.vector.memset(v_sb[h][:, :, :, DH:DH + 1], 1.0)
            o_b = [pp.tile([P, NT, SBLK], BF16, name=f"o{b}") for b in range(NB)]

            # ---- DMAs, in pipeline-priority order ----
            nc.sync.dma_start(bk_sb, bk_d.ap())
            nc.sync.dma_start(wk_sb, wk_d.ap())
            nc.sync.dma_start(xk_sb[0], xk_d[0].ap())
            nc.sync.dma_start(wv_sb, wv_d.ap())
            nc.sync.dma_start(xv_sb[0], xv_d[0].ap())
            nc.sync.dma_start(xk_sb[1], xk_d[1].ap())
            nc.sync.dma_start(bq_sb, bq_d.ap())
            nc.sync.dma_start(wq_sb, wq_d.ap())
            nc.sync.dma_start(xq_sb[0], xq_d[0].ap())
            nc.sync.dma_start(xv_sb[1], xv_d[1].ap())
            nc.sync.dma_start(wo_sb, wo_d.ap())
            nc.sync.dma_start(xq_sb[1], xq_d[1].ap())

            # ---- projection emitters ----
            def proj_qk(x_sb, w_sb, b_sb, dst, blk):
                # one 512-wide s-block of the Q or K projection, [nsl, s] out
                xh = x_sb[blk // 2]
                coff = (blk % 2) * SBLK
                for nt in range(NT):
                    ps = pa.tile([P, SBLK], F32, tag="pa")
                    for kd in range(KD):
                        nc.tensor.matmul(
                            ps,
                            lhsT=w_sb[:, kd, nt * P:(nt + 1) * P],
                            rhs=xh[:, kd, coff:coff + SBLK],
                            start=(kd == 0),
                            stop=(kd == KD - 1),
                        )
                    nc.vector.tensor_scalar_add(dst(nt), ps, b_sb[:, nt:nt + 1])

            def proj_v(st):
                # one 128-row s-tile of the V projection, natural [s, nsl] out
                xh = xv_sb[st // 8]
                coff = (st % 8) * P
                ps = pa.tile([P, SBLK], F32, tag="pa")
                for kd in range(KD):
                    nc.tensor.matmul(
                        ps[:, 0:NSL],
                        lhsT=xh[:, kd, coff:coff + P],
                        rhs=wv_sb[:, kd, :],
                        start=(kd == 0),
                        stop=(kd == KD - 1),
                    )
                nc.vector.tensor_copy(
                    v_sb[st // 8][:, st % 8, :, 0:DH],
                    ps[:, 0:NSL].rearrange("p (h d) -> p h d", d=DH),
                )

            # ---- attention emitters ----
            def scores(ib, hp, jc, ps_s):
                for jj in range(JC):
                    jt = jc * JC + jj
                    kTh = kT[jt // 8]
                    jcol = (jt % 8) * P
                    for h01 in range(2):
                        base = h01 * DH
                        nc.tensor.matmul(
                            ps_s[h01][:, jj, :],
                            lhsT=kTh[base:base + DH, hp, jcol:jcol + P],
                            rhs=qT[ib][base:base + DH, hp, :],
                            start=True,
                            stop=True,
                            tile_position=(base, 0),
                        )

            def exp_chunk(ps_s, p_tiles):
                for h01 in range(2):
                    nc.scalar.activation(
                        p_tiles[h01], ps_s[h01],
                        mybir.ActivationFunctionType.Exp,
                        scale=float(1.0 / np.sqrt(DH)),
                    )

            def pv_chunk(hp, jc, p_tiles, ps_o):
                for h01 in range(2):
                    h = hp * 2 + h01
                    for jj in range(JC):
                        jt = jc * JC + jj
                        nc.tensor.matmul(
                            ps_o[h01][0:DH + 1, :],
                            lhsT=v_sb[jt // 8][:, jt % 8, h, :],
                            rhs=p_tiles[h01][:, jj, :],
                            start=(jt == 0),
                            stop=(jt == JT - 1),
                        )

            def norm(ib, hp, ps_o):
                # o = (exp-weighted V sums) / Z; Z sits in PSUM row DH
                for h01 in range(2):
                    base = h01 * DH
                    zr = pw.tile([1, SBLK], F32, tag="zrow", bufs=2)
                    nc.vector.tensor_copy(zr, ps_o[h01][DH:DH + 1, :])
                    rec = pw.tile([1, SBLK], F32, tag="rec", bufs=2)
                    nc.vector.reciprocal_approx_fast(out=rec, in_=zr)
                    rbc = pw.tile([DH, SBLK], F32, tag="rbc", bufs=2)
                    nc.gpsimd.partition_broadcast(rbc, rec)
                    nc.vector.tensor_mul(
                        o_b[ib][base:base + DH, hp, :], ps_o[h01][0:DH, :], rbc
                    )

            def attention_group(ib, hp):
                ps_o = [
                    psb.tile([P, SBLK], F32, tag=f"ps_o{h01}", bufs=1,
                             name=f"ps_o{h01}")
                    for h01 in range(2)
                ]
                prev = None
                for jc in range(JT // JC):
                    ps_s = [
                        psb.tile([P, JC, SBLK], F32, tag="ps_s", bufs=2,
                                 name=f"ps_s{h01}")
                        for h01 in range(2)
                    ]
                    p_tiles = [
                        pw.tile([P, JC, SBLK], BF16, tag=f"p{h01}", bufs=3,
                                name="p_sb")
                        for h01 in range(2)
                    ]
                    scores(ib, hp, jc, ps_s)
                    exp_chunk(ps_s, p_tiles)
                    if prev is not None:
                        pv_chunk(hp, *prev, ps_o)
                    prev = (jc, p_tiles)
                pv_chunk(hp, *prev, ps_o)
                norm(ib, hp, ps_o)

            def out_proj(ib):
                for st in range(ib * (SBLK // P), (ib + 1) * (SBLK // P)):
                    ob = pw.tile([P, D], F32, tag="ob", bufs=2)
                    ss_off = (st % (SBLK // P)) * P
                    for nb in range(D // SBLK):
                        pso = psb.tile([P, JC, SBLK], F32, tag="ps_s",
                                       bufs=2, name="pso")[:, 0, :]
                        for t in range(NT):
                            nc.tensor.matmul(
                                pso,
                                lhsT=o_b[ib][:, t, ss_off:ss_off + P],
                                rhs=wo_sb[:, t, nb * SBLK:(nb + 1) * SBLK],
                                start=(t == 0),
                                stop=(t == NT - 1),
                            )
                        nc.vector.tensor_copy(
                            ob[:, nb * SBLK:(nb + 1) * SBLK], pso
                        )
                    nc.sync.dma_start(out_d[st * P:(st + 1) * P, :], ob)

            # ---- program order ----
            for blk in range(NB):
                proj_qk(xk_sb, wk_sb, bk_sb,
                        lambda nt, b=blk: kT[b // 2][:, nt,
                                                     (b % 2) * SBLK:
                                                     (b % 2) * SBLK + SBLK],
                        blk)
            proj_qk(xq_sb, wq_sb, bq_sb, lambda nt: qT[0][:, nt, :], 0)
            for st in range(ST):
                proj_v(st)
            for ib in range(NB):
                if ib > 0:
                    proj_qk(xq_sb, wq_sb, bq_sb,
                            lambda nt, b=ib: qT[b][:, nt, :], ib)
                for hp in range(NT):
                    attention_group(ib, hp)
                out_proj(ib)

    nc.finalize()
    return nc


_NC_CACHE = {}


def _get_program():
    if "nc" not in _NC_CACHE:
        _NC_CACHE["nc"] = build_core_program()
    return _NC_CACHE["nc"]


def _marshal_xt(x):
    # [S, D] fp32 -> [P, KD, S] bf16 halves of x^T in SBUF partition layout
    xt = np.ascontiguousarray(x.T).astype(BF)          # [D, S]
    xt = xt.reshape(KD, P, S).transpose(1, 0, 2)       # [P, KD, S]
    return (np.ascontiguousarray(xt[:, :, 0:SH]),
            np.ascontiguousarray(xt[:, :, SH:S]))


def make_in_maps(q, k, v, Wq, bq, Wk, bk, Wv, bv, Wo, bo):
    q, k, v = (np.asarray(x, np.float32) for x in (q, k, v))
    Wq, Wk, Wv, Wo = (np.asarray(x, np.float32) for x in (Wq, Wk, Wv, Wo))
    bq, bk = np.asarray(bq, np.float32), np.asarray(bk, np.float32)
    B = q.shape[0]
    GROUPS = N_CORES // B

    xqs = [_marshal_xt(q[b]) for b in range(B)]
    xks = [_marshal_xt(k[b]) for b in range(B)]
    xvs = [_marshal_xt(v[b]) for b in range(B)]

    in_maps = []
    for c in range(N_CORES):
        b, hg = c // GROUPS, c % GROUPS
        sl = slice(hg * NSL, (hg + 1) * NSL)

        def wslice(W):
            ws = W[:, sl].astype(BF)                      # [D, NSL]
            return np.ascontiguousarray(
                ws.reshape(KD, P, NSL).transpose(1, 0, 2))

        wo_sl = Wo[sl, :].astype(BF)                      # [NSL, D]
        wo_m = np.ascontiguousarray(
            wo_sl.reshape(NT, P, D).transpose(1, 0, 2))

        in_maps.append({
            "xq0": xqs[b][0], "xq1": xqs[b][1],
            "xk0": xks[b][0], "xk1": xks[b][1],
            "xv0": xvs[b][0], "xv1": xvs[b][1],
            "wq": wslice(Wq), "wk": wslice(Wk), "wv": wslice(Wv),
            "wo": wo_m,
            "bq": np.ascontiguousarray(bq[sl].reshape(NT, P).T),
            "bk": np.ascontiguousarray(bk[sl].reshape(NT, P).T),
        })
    return in_maps


def kernel(q, k, v, Wq, bq, Wk, bk, Wv, bv, Wo, bo):
    bv = np.asarray(bv, np.float32)
    bo = np.asarray(bo, np.float32)
    Wo_f = np.asarray(Wo, np.float32)
    B = np.asarray(q).shape[0]
    GROUPS = N_CORES // B

    nc = _get_program()
    in_maps = make_in_maps(q, k, v, Wq, bq, Wk, bk, Wv, bv, Wo, bo)
    res = run_bass_kernel_spmd(nc, in_maps, list(range(N_CORES)))

    out = np.zeros((B, S, D), np.float32)
    for c in range(N_CORES):
        out[c // GROUPS] += res.results[c]["out"]
    # bv commutes through the softmax (weights sum to 1): fold bv@Wo + bo here
    out += (bv @ Wo_f + bo)[None, None, :]
    return out


# revision 6
# speedup vs baseline: 1.3893x; 1.0308x over previous
"""Multi-head attention (B=2, S=2048, D=1024, H=16) on 8 Trainium2 cores.

Sharding: core c = (batch b, head-group hg) with b = c // 4, hg = c % 4.
Each core computes 4 heads of one batch element end-to-end and emits a
partial output projection; the host sums the 4 partials per batch and adds
(bv @ Wo + bo) (the value-bias term commutes through the softmax since the
attention weights sum to 1).

v2 layout strategy (vs the fp32r baseline):
  - Host pre-transposes q/k/v to x^T [D, S] and pre-marshals every tensor
    into its exact SBUF layout ([128, ...] partition-major) in bf16, so all
    DMAs are contiguous-row streams and the 384 PE transposes + PSUM->SBUF
    cast copies disappear entirely.
  - All matmuls run bf16 (full-rate on the PE; accumulation fp32 in PSUM).
  - Softmax denominator comes from a ones-column in V via the PV matmul;
    1/Z is computed with the fast custom-DVE reciprocal straight out of
    PSUM, broadcast across partitions on the otherwise-idle GpSimd engine,
    and applied with one DVE multiply. No PE broadcast, no 3.3us DVE
    reciprocals.
  - Emission order pipelines the whole program: K proj -> Q0 proj ->
    V proj -> per-(i-block, head-pair) attention with in-group
    scores/exp/PV software pipelining, Q(i+1) and the i-block's output
    projection fused between attention groups. ACT streams exp nearly
    continuously from ~35us onward.
"""

import numpy as np
import ml_dtypes

import concourse.bacc as bacc
import concourse.mybir as mybir
import concourse.tile as tile
from concourse.bass_utils import run_bass_kernel_spmd

F32 = mybir.dt.float32
BF16 = mybir.dt.bfloat16
BF = ml_dtypes.bfloat16

S_FULL, D_FULL, NH_PER_CORE, DH = 2048, 1024, 4, 64
N_CORES = 8
B_FULL, H_FULL = 2, 16

P = 128
S, D, NH = S_FULL, D_FULL, NH_PER_CORE
NSL = NH * DH            # 256: projection slice width per core
KD = D // P              # 8 contraction tiles
NT = NSL // P            # 2 head-pairs
ST = S // P              # 16 s-tiles
SBLK = 512               # i-block width
NB = S // SBLK           # 4 i-blocks
JT = ST                  # 16 j-tiles
JC = 2                   # j-tiles per score/exp chunk
SH = S // 2              # half-sequence (DMA/pipeline granularity)


def build_core_program():
    nc = bacc.Bacc("TRN2", target_bir_lowering=False, debug=False)

    xq_d = [nc.dram_tensor(f"xq{h}", [P, KD, SH], BF16, kind="ExternalInput")
            for h in range(2)]
    xk_d = [nc.dram_tensor(f"xk{h}", [P, KD, SH], BF16, kind="ExternalInput")
            for h in range(2)]
    xv_d = [nc.dram_tensor(f"xv{h}", [P, KD, SH], BF16, kind="ExternalInput")
            for h in range(2)]
    wq_d = nc.dram_tensor("wq", [P, KD, NSL], BF16, kind="ExternalInput")
    wk_d = nc.dram_tensor("wk", [P, KD, NSL], BF16, kind="ExternalInput")
    wv_d = nc.dram_tensor("wv", [P, KD, NSL], BF16, kind="ExternalInput")
    wo_d = nc.dram_tensor("wo", [P, NT, D], BF16, kind="ExternalInput")
    bq_d = nc.dram_tensor("bq", [P, NT], F32, kind="ExternalInput")
    bk_d = nc.dram_tensor("bk", [P, NT], F32, kind="ExternalInput")
    out_d = nc.dram_tensor("out", [S, D], F32, kind="ExternalOutput")

    with tile.TileContext(nc) as tc:
        with tc.tile_pool(name="persist", bufs=1) as pp, \
             tc.tile_pool(name="work", bufs=2) as pw, \
             tc.tile_pool(name="pa", bufs=2, space="PSUM") as pa, \
             tc.tile_pool(name="pb", bufs=1, space="PSUM") as psb:

            # ---- persistent SBUF tensors ----
            wq_sb = pp.tile([P, KD, NSL], BF16, name="wq")
            wk_sb = pp.tile([P, KD, NSL], BF16, name="wk")
            wv_sb = pp.tile([P, KD, NSL], BF16, name="wv")
            wo_sb = pp.tile([P, NT, D], BF16, name="wo")
            bq_sb = pp.tile([P, NT], F32, name="bq")
            bk_sb = pp.tile([P, NT], F32, name="bk")
            xq_sb = [pp.tile([P, KD, SH], BF16, name=f"xq{h}") for h in range(2)]
            xk_sb = [pp.tile([P, KD, SH], BF16, name=f"xk{h}") for h in range(2)]
            xv_sb = [pp.tile([P, KD, SH], BF16, name=f"xv{h}") for h in range(2)]
            kT = [pp.tile([P, NT, SH], BF16, name=f"kT{h}") for h in range(2)]
            qT = [pp.tile([P, NT, SBLK], BF16, name=f"qT{b}") for b in range(NB)]
            # natural-layout V (+ ones column feeding the softmax denominator)
            v_sb = [pp.tile([P, JT // 2, NH, DH + 1], BF16, name=f"v{h}")
                    for h in range(2)]
            for h in range(2):
                nc.vector.memset(v_sb[h][:, :, :, DH:DH + 1], 1.0)
            o_b = [pp.tile([P, NT, SBLK], BF16, name=f"o{b}") for b in range(NB)]

            # ---- DMAs, in pipeline-priority order ----
            nc.sync.dma_start(bk_sb, bk_d.ap())
            nc.sync.dma_start(wk_sb, wk_d.ap())
            nc.sync.dma_start(xk_sb[0], xk_d[0].ap())
            nc.sync.dma_start(wv_sb, wv_d.ap())
            nc.sync.dma_start(xv_sb[0], xv_d[0].ap())
            nc.sync.dma_start(xk_sb[1], xk_d[1].ap())
            nc.sync.dma_start(bq_sb, bq_d.ap())
            nc.sync.dma_start(wq_sb, wq_d.ap())
            nc.sync.dma_start(xq_sb[0], xq_d[0].ap())
            nc.sync.dma_start(xv_sb[1], xv_d[1].ap())
            nc.sync.dma_start(wo_sb, wo_d.ap())
            nc.sync.dma_start(xq_sb[1], xq_d[1].ap())

            # ---- projection emitters ----
            def proj_qk_nt(x_sb, w_sb, b_sb, dst_ap, blk, nt):
                # one [128, 512] tile of the Q or K projection, [nsl, s] out
                xh = x_sb[blk // 2]
                coff = (blk % 2) * SBLK
                ps = pa.tile([P, SBLK], F32, tag="pa")
                for kd in range(KD):
                    nc.tensor.matmul(
                        ps,
                        lhsT=w_sb[:, kd, nt * P:(nt + 1) * P],
                        rhs=xh[:, kd, coff:coff + SBLK],
                        start=(kd == 0),
                        stop=(kd == KD - 1),
                    )
                nc.vector.tensor_scalar_add(dst_ap, ps, b_sb[:, nt:nt + 1])

            def proj_v(st):
                # one 128-row s-tile of the V projection, natural [s, nsl] out
                xh = xv_sb[st // 8]
                coff = (st % 8) * P
                ps = pa.tile([P, SBLK], F32, tag="pa")
                for kd in range(KD):
                    nc.tensor.matmul(
                        ps[:, 0:NSL],
                        lhsT=xh[:, kd, coff:coff + P],
                        rhs=wv_sb[:, kd, :],
                        start=(kd == 0),
                        stop=(kd == KD - 1),
                    )
                nc.vector.tensor_copy(
                    v_sb[st // 8][:, st % 8, :, 0:DH],
                    ps[:, 0:NSL].rearrange("p (h d) -> p h d", d=DH),
                )

            # ---- attention emitters ----
            def scores(ib, hp, jc, ps_s):
                for jj in range(JC):
                    jt = jc * JC + jj
                    kTh = kT[jt // 8]
                    jcol = (jt % 8) * P
                    for h01 in range(2):
                        base = h01 * DH
                        nc.tensor.matmul(
                            ps_s[h01][:, jj, :],
                            lhsT=kTh[base:base + DH, hp, jcol:jcol + P],
                            rhs=qT[ib][base:base + DH, hp, :],
                            start=True,
                            stop=True,
                            tile_position=(base, 0),
                        )

            def exp_chunk(ps_s, p_tiles):
                for h01 in range(2):
                    nc.scalar.activation(
                        p_tiles[h01], ps_s[h01],
                        mybir.ActivationFunctionType.Exp,
                        scale=float(1.0 / np.sqrt(DH)),
                    )

            def pv_chunk(hp, jc, p_tiles, ps_o):
                for h01 in range(2):
                    h = hp * 2 + h01
                    for jj in range(JC):
                        jt = jc * JC + jj
                        nc.tensor.matmul(
                            ps_o[h01][0:DH + 1, :],
                            lhsT=v_sb[jt // 8][:, jt % 8, h, :],
                            rhs=p_tiles[h01][:, jj, :],
                            start=(jt == 0),
                            stop=(jt == JT - 1),
                        )

            def norm(ib, hp, ps_o):
                # o = (exp-weighted V sums) / Z; Z sits in PSUM row DH
                for h01 in range(2):
                    base = h01 * DH
                    zr = pw.tile([1, SBLK], F32, tag="zrow", bufs=2)
                    nc.vector.tensor_copy(zr, ps_o[h01][DH:DH + 1, :])
                    rec = pw.tile([1, SBLK], F32, tag="rec", bufs=2)
                    nc.vector.reciprocal_approx_fast(out=rec, in_=zr)
                    rbc = pw.tile([DH, SBLK], F32, tag="rbc", bufs=2)
                    nc.gpsimd.partition_broadcast(rbc, rec)
                    nc.vector.tensor_mul(
                        o_b[ib][base:base + DH, hp, :], ps_o[h01][0:DH, :], rbc
                    )

            def out_proj_st(ib, st):
                # one 128-row output tile: both D halves + staging copy + DMA
                ob = pw.tile([P, D], F32, tag="ob", bufs=2)
                ss_off = (st % (SBLK // P)) * P
                for nb in range(D // SBLK):
                    pso = psb.tile([P, JC, SBLK], F32, tag="ps_s",
                                   bufs=2, name="pso")[:, 0, :]
                    for t in range(NT):
                        nc.tensor.matmul(
                            pso,
                            lhsT=o_b[ib][:, t, ss_off:ss_off + P],
                            rhs=wo_sb[:, t, nb * SBLK:(nb + 1) * SBLK],
                            start=(t == 0),
                            stop=(t == NT - 1),
                        )
                    nc.vector.tensor_copy(ob[:, nb * SBLK:(nb + 1) * SBLK], pso)
                nc.sync.dma_start(out_d[st * P:(st + 1) * P, :], ob)

            # ---- program order ----
            # PE filler queue: each thunk is ~0.9us of PE work, pumped one
            # per attention chunk to keep the PE dense (ramped clock) while
            # ACT streams exp at ~2.2us/chunk.
            fillers = []

            def pump(n=1):
                for _ in range(n):
                    if fillers:
                        fillers.pop(0)()

            for blk in range(NB):
                for nt in range(NT):
                    proj_qk_nt(
                        xk_sb, wk_sb, bk_sb,
                        kT[blk // 2][:, nt, (blk % 2) * SBLK:
                                     (blk % 2) * SBLK + SBLK],
                        blk, nt)
            for nt in range(NT):
                proj_qk_nt(xq_sb, wq_sb, bq_sb, qT[0][:, nt, :], 0, nt)
            for st in range(8):
                proj_v(st)
            fillers += [(lambda s=st: proj_v(s)) for st in range(8, ST)]

            for ib in range(NB):
                for hp in range(NT):
                    if hp == 1 and ib < NB - 1:
                        # Q projection for the next i-block (xq half arrives
                        # early; emitted here so PE meets it ramped)
                        fillers += [
                            (lambda b=ib + 1, nt=nt:
                             proj_qk_nt(xq_sb, wq_sb, bq_sb,
                                        qT[b][:, nt, :], b, nt))
                            for nt in range(NT)
                        ]
                    ps_o = [
                        psb.tile([P, SBLK], F32, tag=f"ps_o{h01}", bufs=1,
                                 name=f"ps_o{h01}")
                        for h01 in range(2)
                    ]
                    prev = None
                    for jc in range(JT // JC):
                        ps_s = [
                            psb.tile([P, JC, SBLK], F32, tag="ps_s", bufs=2,
                                     name=f"ps_s{h01}")
                            for h01 in range(2)
                        ]
                        p_tiles = [
                            pw.tile([P, JC, SBLK], BF16, tag=f"p{h01}",
                                    bufs=3, name="p_sb")
                            for h01 in range(2)
                        ]
                        scores(ib, hp, jc, ps_s)
                        exp_chunk(ps_s, p_tiles)
                        if prev is not None:
                            pv_chunk(hp, *prev, ps_o)
                        pump(1)
                        prev = (jc, p_tiles)
                    pv_chunk(hp, *prev, ps_o)
                    norm(ib, hp, ps_o)
                    if hp == 1:
                        fillers += [
                            (lambda b=ib, s=st: out_proj_st(b, s))
                            for st in range(ib * (SBLK // P),
                                            (ib + 1) * (SBLK // P))
                        ]
            pump(len(fillers))

    nc.finalize()
    return nc


_NC_CACHE = {}


def _get_program():
    if "nc" not in _NC_CACHE:
        _NC_CACHE["nc"] = build_core_program()
    return _NC_CACHE["nc"]


def _marshal_xt(x):
    # [S, D] fp32 -> [P, KD, S] bf16 halves of x^T in SBUF partition layout
    xt = np.ascontiguousarray(x.T).astype(BF)          # [D, S]
    xt = xt.reshape(KD, P, S).transpose(1, 0, 2)       # [P, KD, S]
    return (np.ascontiguousarray(xt[:, :, 0:SH]),
            np.ascontiguousarray(xt[:, :, SH:S]))


def make_in_maps(q, k, v, Wq, bq, Wk, bk, Wv, bv, Wo, bo):
    q, k, v = (np.asarray(x, np.float32) for x in (q, k, v))
    Wq, Wk, Wv, Wo = (np.asarray(x, np.float32) for x in (Wq, Wk, Wv, Wo))
    bq, bk = np.asarray(bq, np.float32), np.asarray(bk, np.float32)
    B = q.shape[0]
    GROUPS = N_CORES // B

    xqs = [_marshal_xt(q[b]) for b in range(B)]
    xks = [_marshal_xt(k[b]) for b in range(B)]
    xvs = [_marshal_xt(v[b]) for b in range(B)]

    in_maps = []
    for c in range(N_CORES):
        b, hg = c // GROUPS, c % GROUPS
        sl = slice(hg * NSL, (hg + 1) * NSL)

        def wslice(W):
            ws = W[:, sl].astype(BF)                      # [D, NSL]
            return np.ascontiguousarray(
                ws.reshape(KD, P, NSL).transpose(1, 0, 2))

        wo_sl = Wo[sl, :].astype(BF)                      # [NSL, D]
        wo_m = np.ascontiguousarray(
            wo_sl.reshape(NT, P, D).transpose(1, 0, 2))

        in_maps.append({
            "xq0": xqs[b][0], "xq1": xqs[b][1],
            "xk0": xks[b][0], "xk1": xks[b][1],
            "xv0": xvs[b][0], "xv1": xvs[b][1],
            "wq": wslice(Wq), "wk": wslice(Wk), "wv": wslice(Wv),
            "wo": wo_m,
            "bq": np.ascontiguousarray(bq[sl].reshape(NT, P).T),
            "bk": np.ascontiguousarray(bk[sl].reshape(NT, P).T),
        })
    return in_maps


def kernel(q, k, v, Wq, bq, Wk, bk, Wv, bv, Wo, bo):
    bv = np.asarray(bv, np.float32)
    bo = np.asarray(bo, np.float32)
    Wo_f = np.asarray(Wo, np.float32)
    B = np.asarray(q).shape[0]
    GROUPS = N_CORES // B

    nc = _get_program()
    in_maps = make_in_maps(q, k, v, Wq, bq, Wk, bk, Wv, bv, Wo, bo)
    res = run_bass_kernel_spmd(nc, in_maps, list(range(N_CORES)))

    out = np.zeros((B, S, D), np.float32)
    for c in range(N_CORES):
        out[c // GROUPS] += res.results[c]["out"]
    # bv commutes through the softmax (weights sum to 1): fold bv@Wo + bo here
    out += (bv @ Wo_f + bo)[None, None, :]
    return out


# revision 7
# speedup vs baseline: 1.7170x; 1.2359x over previous
"""Multi-head attention (B=2, S=2048, D=1024, H=16) on 8 Trainium2 cores.

Sharding: core c = (batch b, head-group hg) with b = c // 4, hg = c % 4.
Each core computes 4 heads of one batch element end-to-end and emits a
partial output projection; the host sums the 4 partials per batch and adds
(bv @ Wo + bo) (the value-bias term commutes through the softmax since the
attention weights sum to 1).

v2 layout strategy (vs the fp32r baseline):
  - Host pre-transposes q/k/v to x^T [D, S] and pre-marshals every tensor
    into its exact SBUF layout ([128, ...] partition-major) in bf16, so all
    DMAs are contiguous-row streams and the 384 PE transposes + PSUM->SBUF
    cast copies disappear entirely.
  - All matmuls run bf16 (full-rate on the PE; accumulation fp32 in PSUM).
  - Softmax denominator comes from a ones-column in V via the PV matmul;
    1/Z is computed with the fast custom-DVE reciprocal straight out of
    PSUM, broadcast across partitions on the otherwise-idle GpSimd engine,
    and applied with one DVE multiply. No PE broadcast, no 3.3us DVE
    reciprocals.
  - Emission order pipelines the whole program: K proj -> Q0 proj ->
    V proj -> per-(i-block, head-pair) attention with in-group
    scores/exp/PV software pipelining, Q(i+1) and the i-block's output
    projection fused between attention groups. ACT streams exp nearly
    continuously from ~35us onward.
"""

import numpy as np
import ml_dtypes

import concourse.bacc as bacc
import concourse.mybir as mybir
import concourse.tile as tile
from concourse.bass_utils import run_bass_kernel_spmd

F32 = mybir.dt.float32
BF16 = mybir.dt.bfloat16
BF = ml_dtypes.bfloat16

S_FULL, D_FULL, NH_PER_CORE, DH = 2048, 1024, 4, 64
N_CORES = 8
B_FULL, H_FULL = 2, 16

P = 128
S, D, NH = S_FULL, D_FULL, NH_PER_CORE
NSL = NH * DH            # 256: projection slice width per core
KD = D // P              # 8 contraction tiles
NT = NSL // P            # 2 head-pairs
ST = S // P              # 16 s-tiles
SBLK = 512               # i-block width
NB = S // SBLK           # 4 i-blocks
JT = ST                  # 16 j-tiles
JC = 2                   # j-tiles per score/exp chunk
SH = S // 2              # half-sequence (DMA/pipeline granularity)


def build_core_program():
    nc = bacc.Bacc("TRN2", target_bir_lowering=False, debug=False)

    xq_d = [nc.dram_tensor(f"xq{h}", [P, KD, SH], BF16, kind="ExternalInput")
            for h in range(2)]
    xk_d = [nc.dram_tensor(f"xk{h}", [P, KD, SH], BF16, kind="ExternalInput")
            for h in range(2)]
    xv_d = [nc.dram_tensor(f"xv{h}", [P, KD, SH], BF16, kind="ExternalInput")
            for h in range(2)]
    wq_d = nc.dram_tensor("wq", [P, KD, NSL], BF16, kind="ExternalInput")
    wk_d = nc.dram_tensor("wk", [P, KD, NSL], BF16, kind="ExternalInput")
    wv_d = nc.dram_tensor("wv", [P, KD, NSL], BF16, kind="ExternalInput")
    wo_d = nc.dram_tensor("wo", [P, NT, D], BF16, kind="ExternalInput")
    bq_d = nc.dram_tensor("bq", [P, NT], F32, kind="ExternalInput")
    bk_d = nc.dram_tensor("bk", [P, NT], F32, kind="ExternalInput")
    out_d = nc.dram_tensor("out", [S, D], F32, kind="ExternalOutput")

    with tile.TileContext(nc) as tc:
        with tc.tile_pool(name="persist", bufs=1) as pp, \
             tc.tile_pool(name="work", bufs=2) as pw, \
             tc.tile_pool(name="pa", bufs=1, space="PSUM") as pa, \
             tc.tile_pool(name="pb", bufs=1, space="PSUM") as psb:

            # ---- persistent SBUF tensors ----
            wq_sb = pp.tile([P, KD, NSL], BF16, name="wq")
            wk_sb = pp.tile([P, KD, NSL], BF16, name="wk")
            wv_sb = pp.tile([P, KD, NSL], BF16, name="wv")
            wo_sb = pp.tile([P, NT, D], BF16, name="wo")
            bq_sb = pp.tile([P, NT], F32, name="bq")
            bk_sb = pp.tile([P, NT], F32, name="bk")
            xq_sb = [pp.tile([P, KD, SH], BF16, name=f"xq{h}") for h in range(2)]
            xk_sb = [pp.tile([P, KD, SH], BF16, name=f"xk{h}") for h in range(2)]
            xv_sb = [pp.tile([P, KD, SH], BF16, name=f"xv{h}") for h in range(2)]
            kT = [pp.tile([P, NT, SH], BF16, name=f"kT{h}") for h in range(2)]
            qT = [pp.tile([P, NT, SBLK], BF16, name=f"qT{b}") for b in range(NB)]
            # natural-layout V (+ ones column feeding the softmax denominator)
            v_sb = [pp.tile([P, JT // 2, NH, DH + 1], BF16, name=f"v{h}")
                    for h in range(2)]
            for h in range(2):
                nc.vector.memset(v_sb[h][:, :, :, DH:DH + 1], 1.0)
            o_b = [pp.tile([P, NT, SBLK], BF16, name=f"o{b}") for b in range(NB)]

            # ---- DMAs, in pipeline-priority order ----
            nc.sync.dma_start(bk_sb, bk_d.ap())
            nc.sync.dma_start(wk_sb, wk_d.ap())
            nc.sync.dma_start(xk_sb[0], xk_d[0].ap())
            nc.sync.dma_start(wv_sb, wv_d.ap())
            nc.sync.dma_start(xv_sb[0], xv_d[0].ap())
            nc.sync.dma_start(xk_sb[1], xk_d[1].ap())
            nc.sync.dma_start(bq_sb, bq_d.ap())
            nc.sync.dma_start(wq_sb, wq_d.ap())
            nc.sync.dma_start(xq_sb[0], xq_d[0].ap())
            nc.sync.dma_start(xv_sb[1], xv_d[1].ap())
            nc.sync.dma_start(wo_sb, wo_d.ap())
            nc.sync.dma_start(xq_sb[1], xq_d[1].ap())

            # ---- projection emitters ----
            def proj_qk_nt(x_sb, w_sb, b_sb, dst_ap, blk, nt, lead=False):
                # one [128, 512] tile of the Q or K projection, [nsl, s] out
                xh = x_sb[blk // 2]
                coff = (blk % 2) * SBLK
                if lead:
                    ps = psb.tile([P, JC, SBLK], F32, tag="ps_s", bufs=2,
                                  name="ps_proj")[:, 0, :]
                else:
                    ps = pa.tile([P, SBLK], F32, tag="pa")
                for kd in range(KD):
                    nc.tensor.matmul(
                        ps,
                        lhsT=w_sb[:, kd, nt * P:(nt + 1) * P],
                        rhs=xh[:, kd, coff:coff + SBLK],
                        start=(kd == 0),
                        stop=(kd == KD - 1),
                    )
                nc.vector.tensor_scalar_add(dst_ap, ps, b_sb[:, nt:nt + 1])

            def proj_v(st):
                # one 128-row s-tile of the V projection, natural [s, nsl] out
                xh = xv_sb[st // 8]
                coff = (st % 8) * P
                ps = pa.tile([P, SBLK], F32, tag="pa")
                for kd in range(KD):
                    nc.tensor.matmul(
                        ps[:, 0:NSL],
                        lhsT=xh[:, kd, coff:coff + P],
                        rhs=wv_sb[:, kd, :],
                        start=(kd == 0),
                        stop=(kd == KD - 1),
                    )
                nc.vector.tensor_copy(
                    v_sb[st // 8][:, st % 8, :, 0:DH],
                    ps[:, 0:NSL].rearrange("p (h d) -> p h d", d=DH),
                )

            # ---- attention emitters ----
            def scores(ib, hp, jc, ps_s):
                for jj in range(JC):
                    jt = jc * JC + jj
                    kTh = kT[jt // 8]
                    jcol = (jt % 8) * P
                    for h01 in range(2):
                        base = h01 * DH
                        nc.tensor.matmul(
                            ps_s[h01][:, jj, :],
                            lhsT=kTh[base:base + DH, hp, jcol:jcol + P],
                            rhs=qT[ib][base:base + DH, hp, :],
                            start=True,
                            stop=True,
                            tile_position=(base, 0),
                        )

            def exp_chunk(ps_s, p_tiles):
                for h01 in range(2):
                    nc.scalar.activation(
                        p_tiles[h01], ps_s[h01],
                        mybir.ActivationFunctionType.Exp,
                        scale=float(1.0 / np.sqrt(DH)),
                    )

            def pv_chunk(hp, jc, p_tiles, ps_o):
                for h01 in range(2):
                    h = hp * 2 + h01
                    for jj in range(JC):
                        jt = jc * JC + jj
                        nc.tensor.matmul(
                            ps_o[h01][0:DH + 1, :],
                            lhsT=v_sb[jt // 8][:, jt % 8, h, :],
                            rhs=p_tiles[h01][:, jj, :],
                            start=(jt == 0),
                            stop=(jt == JT - 1),
                        )

            def norm(ib, hp, ps_o):
                # o = (exp-weighted V sums) / Z; Z sits in PSUM row DH
                for h01 in range(2):
                    base = h01 * DH
                    zr = pw.tile([1, SBLK], F32, tag="zrow", bufs=2)
                    nc.vector.tensor_copy(zr, ps_o[h01][DH:DH + 1, :])
                    rec = pw.tile([1, SBLK], F32, tag="rec", bufs=2)
                    nc.vector.reciprocal_approx_fast(out=rec, in_=zr)
                    rbc = pw.tile([DH, SBLK], F32, tag="rbc", bufs=2)
                    nc.gpsimd.partition_broadcast(rbc, rec)
                    nc.vector.tensor_mul(
                        o_b[ib][base:base + DH, hp, :], ps_o[h01][0:DH, :], rbc
                    )

            def out_proj_st(ib, st):
                # one 128-row output tile: both D halves + staging copy + DMA
                ob = pw.tile([P, D], F32, tag="ob", bufs=2)
                ss_off = (st % (SBLK // P)) * P
                for nb in range(D // SBLK):
                    pso = pa.tile([P, SBLK], F32, tag="pa", name="pso")
                    for t in range(NT):
                        nc.tensor.matmul(
                            pso,
                            lhsT=o_b[ib][:, t, ss_off:ss_off + P],
                            rhs=wo_sb[:, t, nb * SBLK:(nb + 1) * SBLK],
                            start=(t == 0),
                            stop=(t == NT - 1),
                        )
                    nc.vector.tensor_copy(ob[:, nb * SBLK:(nb + 1) * SBLK], pso)
                nc.sync.dma_start(out_d[st * P:(st + 1) * P, :], ob)

            # ---- program order ----
            # PE filler queue: each thunk is ~0.9us of PE work, pumped one
            # per attention chunk to keep the PE dense (ramped clock) while
            # ACT streams exp at ~2.2us/chunk.
            fillers = []

            def pump(n=1):
                for _ in range(n):
                    if fillers:
                        fillers.pop(0)()

            for blk in range(NB):
                for nt in range(NT):
                    proj_qk_nt(
                        xk_sb, wk_sb, bk_sb,
                        kT[blk // 2][:, nt, (blk % 2) * SBLK:
                                     (blk % 2) * SBLK + SBLK],
                        blk, nt, lead=True)
            for nt in range(NT):
                proj_qk_nt(xq_sb, wq_sb, bq_sb, qT[0][:, nt, :], 0, nt,
                           lead=True)
            fillers += [(lambda s=st: proj_v(s)) for st in range(ST)]

            for ib in range(NB):
                for hp in range(NT):
                    if hp == 1 and ib < NB - 1:
                        # Q projection for the next i-block (xq half arrives
                        # early; emitted here so PE meets it ramped)
                        fillers += [
                            (lambda b=ib + 1, nt=nt:
                             proj_qk_nt(xq_sb, wq_sb, bq_sb,
                                        qT[b][:, nt, :], b, nt))
                            for nt in range(NT)
                        ]
                    ps_o = [
                        psb.tile([P, SBLK], F32, tag="ps_o", bufs=3,
                                 name=f"ps_o{h01}")
                        for h01 in range(2)
                    ]
                    first = (ib == 0 and hp == 0)
                    lag = 4 if first else 1
                    pending = []
                    for jc in range(JT // JC):
                        ps_s = [
                            psb.tile([P, JC, SBLK], F32, tag="ps_s", bufs=2,
                                     name=f"ps_s{h01}")
                            for h01 in range(2)
                        ]
                        p_tiles = [
                            pw.tile([P, JC, SBLK], BF16, tag=f"p{h01}",
                                    bufs=5, name="p_sb")
                            for h01 in range(2)
                        ]
                        scores(ib, hp, jc, ps_s)
                        exp_chunk(ps_s, p_tiles)
                        pending.append((jc, p_tiles))
                        if len(pending) > lag:
                            pv_chunk(hp, *pending.pop(0), ps_o)
                        if first:
                            pump(2)
                        elif jc >= 2:
                            pump(1)
                    for item in pending:
                        pv_chunk(hp, *item, ps_o)
                    norm(ib, hp, ps_o)
                    if hp == 1:
                        fillers += [
                            (lambda b=ib, s=st: out_proj_st(b, s))
                            for st in range(ib * (SBLK // P),
                                            (ib + 1) * (SBLK // P))
                        ]
            pump(len(fillers))

    nc.finalize()
    return nc


_NC_CACHE = {}


def _get_program():
    if "nc" not in _NC_CACHE:
        _NC_CACHE["nc"] = build_core_program()
    return _NC_CACHE["nc"]


def _marshal_xt(x):
    # [S, D] fp32 -> [P, KD, S] bf16 halves of x^T in SBUF partition layout
    xt = np.ascontiguousarray(x.T).astype(BF)          # [D, S]
    xt = xt.reshape(KD, P, S).transpose(1, 0, 2)       # [P, KD, S]
    return (np.ascontiguousarray(xt[:, :, 0:SH]),
            np.ascontiguousarray(xt[:, :, SH:S]))


def make_in_maps(q, k, v, Wq, bq, Wk, bk, Wv, bv, Wo, bo):
    q, k, v = (np.asarray(x, np.float32) for x in (q, k, v))
    Wq, Wk, Wv, Wo = (np.asarray(x, np.float32) for x in (Wq, Wk, Wv, Wo))
    bq, bk = np.asarray(bq, np.float32), np.asarray(bk, np.float32)
    B = q.shape[0]
    GROUPS = N_CORES // B

    xqs = [_marshal_xt(q[b]) for b in range(B)]
    xks = [_marshal_xt(k[b]) for b in range(B)]
    xvs = [_marshal_xt(v[b]) for b in range(B)]

    in_maps = []
    for c in range(N_CORES):
        b, hg = c // GROUPS, c % GROUPS
        sl = slice(hg * NSL, (hg + 1) * NSL)

        def wslice(W):
            ws = W[:, sl].astype(BF)                      # [D, NSL]
            return np.ascontiguousarray(
                ws.reshape(KD, P, NSL).transpose(1, 0, 2))

        wo_sl = Wo[sl, :].astype(BF)                      # [NSL, D]
        wo_m = np.ascontiguousarray(
            wo_sl.reshape(NT, P, D).transpose(1, 0, 2))

        in_maps.append({
            "xq0": xqs[b][0], "xq1": xqs[b][1],
            "xk0": xks[b][0], "xk1": xks[b][1],
            "xv0": xvs[b][0], "xv1": xvs[b][1],
            "wq": wslice(Wq), "wk": wslice(Wk), "wv": wslice(Wv),
            "wo": wo_m,
            "bq": np.ascontiguousarray(bq[sl].reshape(NT, P).T),
            "bk": np.ascontiguousarray(bk[sl].reshape(NT, P).T),
        })
    return in_maps


def kernel(q, k, v, Wq, bq, Wk, bk, Wv, bv, Wo, bo):
    bv = np.asarray(bv, np.float32)
    bo = np.asarray(bo, np.float32)
    Wo_f = np.asarray(Wo, np.float32)
    B = np.asarray(q).shape[0]
    GROUPS = N_CORES // B

    nc = _get_program()
    in_maps = make_in_maps(q, k, v, Wq, bq, Wk, bk, Wv, bv, Wo, bo)
    res = run_bass_kernel_spmd(nc, in_maps, list(range(N_CORES)))

    out = np.zeros((B, S, D), np.float32)
    for c in range(N_CORES):
        out[c // GROUPS] += res.results[c]["out"]
    # bv commutes through the softmax (weights sum to 1): fold bv@Wo + bo here
    out += (bv @ Wo_f + bo)[None, None, :]
    return out
